# revision 3
# baseline (speedup 1.0000x reference)
"""Trainium2 Bass kernel for DualThresholdSelfregulatingIntegrate (v4).

Math identical to the reference decomposition: rates = relu(x)*dt on the
engines, chunk-local cumsum + [t,d]->[d,t'] transpose fused in one fp32 PE
matmul per 128x128 block (L = rates^T @ triu_ones), Kahan-compensated fp32
carries across chunks (chain starts at v0 - 0.5), and F = rint(L + carry)
= floor(c) emitted per step as int8.

The device ships F (the per-step floor counts, int8, [b, d, t] layout);
the host applies the exact integer adjacent-difference (spikes = F_t -
F_{t-1}, with floor(v0) = 0), the layout transpose, and the f32 1/dt
scale. Shipping F instead of the diff halves output HBM bytes vs int16
spikes (device DMA total 21 MB/core vs 25.2) and removes the diff pass +
chunk-boundary bookkeeping (~20 us of DVE) entirely; the int diff is
bit-exact so the result is unchanged (2 bitwise mismatches vs the jax
reference, identical to the int16-spike baseline).

Engine assignment (per the CoreSim cost model, validated on HW):
  - PE: 256 normal-mode fp32 matmuls (transpose-mode is 2x faster in the
    model but numerically WRONG on HW - do not enable "tmm")
  - DVE: F STT (PSUM -> int8, carry broadcast along t'), Kahan chain
    (high priority - they gate PSUM recycle), 12/32 of relu as
    tensor_scalar max(x,0)*dt (bitwise == ScalarE Relu(x*dt))
  - ScalarE: 20/32 of relu (the 2.3x SBUF-src errata makes it slow;
    keeping all relu here was the old kernel's main bottleneck)
  - GpSimd/Pool: DMA queue only. Pool tensor ops are ~8 us each on HW
    (Q7 dispatch) and integer subtract is unsupported - never compute on
    Pool
  - DMA: joint-batch group loads (1 MB, SP/Pool HWDGE+SWDGE rings),
    whole-group int8 F stores; the 'a' (ACT) ring is avoided because DMA
    triggers queue behind multi-us relu ops in the ACT FIFO
  - bench loop: the body is unrolled 8x inside tc.For_i so the loop
    back-edge sync (~9 us) amortizes and the pipeline flows across
    bodies; the single-shot build (bench_iters=0) is one body
"""

import sys

sys.path.insert(0, "/opt/trn_rl_repo")

import numpy as np

import concourse.bass as bass  # noqa: F401  (registers engines)
import concourse.tile as tile
from concourse import bacc, mybir

N_CORES = 8
B, T, D = 16, 2048, 1024
BC = B // N_CORES          # batches per core
CH = 128                   # chunk (carry granularity, matmul contraction)
G = 512                    # group: time steps per pipeline stage
CPG = G // CH              # 4 chunks per group
NG = T // G                # 4 groups per batch
NDB = D // CH              # 8 d-blocks
dt = mybir.dt

_cache = {}
CFG = {
    "tmm": 0,          # 1 = transpose-mode fp32 matmuls (2 cyc/row vs 4)
    "rle": "aavaavav",  # relu engine per (b,c) unit, cycled: v=DVE g=Pool a=ACT
    "dfe": "g",        # diff engine per chunk, cycled
    "ldn": 2,          # (b,chunk) units (CH x D fp32) per load DMA: 4 -> 2.1 MB
    "stn": 8,          # (b,chunk) units per store DMA: 8 -> whole group
    "odt": "hf8",      # i8 | i16c8 (int16 spikes, SWDGE cast store) | i16
    "pcb": 2,          # pc PSUM bufs (4 banks each, joint-batch)
    "ldq": "ssgssgsg",  # per-load queue cycle: s=sync(SP), a=scalar(ACT), g=pool
    "stq": "sg",       # per-store queue cycle
    "xbuf": 2,         # xin/rt pool bufs
    "unroll": 8,       # body repetitions inside the For_i loop
    "pke": "vg",       # pack-op engine cycle for odt=p2
    "rlp": 0,          # deprioritize relu by this much (0 = program order)
}
ENGQ = {"v": "vector", "g": "gpsimd", "s": "sync", "a": "scalar"}


def build_nc(bench_iters=0, skip=(), cfg=None):
    if cfg:
        CFG.update(cfg)
    if CFG["odt"] == "f8":
        odt = sdt = dt.float8e4
    else:
        odt = dt.int8 if CFG["odt"] in ("i8", "i16c8", "p2", "hf8") else dt.int16
        sdt = dt.int16 if CFG["odt"] in ("i16c8", "p2") else odt
    yt = T // 4 if CFG["odt"] == "p2" else T
    nc = bacc.Bacc("TRN2", target_bir_lowering=False, debug=False)
    x = nc.dram_tensor("x", [BC, T, D], dt.float32, kind="ExternalInput")
    v0t = nc.dram_tensor("v0t", [BC, CH, NDB], dt.float32, kind="ExternalInput")
    u = nc.dram_tensor("u", [CH, CH], dt.float32, kind="ExternalInput")
    y = nc.dram_tensor("y", [BC, D, yt], odt, kind="ExternalOutput")

    with tile.TileContext(nc) as tc:
        with tc.tile_pool(name="xin", bufs=CFG["xbuf"]) as xin_p, \
             tc.tile_pool(name="rt", bufs=CFG["xbuf"]) as rt_p, \
             tc.tile_pool(name="ff", bufs=2) as f_p, \
             tc.tile_pool(name="sp", bufs=2) as sp_p, \
             tc.tile_pool(name="cr", bufs=2) as cr_p, \
             tc.tile_pool(name="sm", bufs=3) as sm_p, \
             tc.tile_pool(name="consts", bufs=1) as c_p, \
             tc.tile_pool(name="pc", bufs=CFG["pcb"], space="PSUM") as pc_p:

            ut = c_p.tile([CH, CH], dt.float32, tag="ut")
            nc.sync.dma_start(ut[:], u[:])
            v0tt = c_p.tile([CH, BC * NDB], dt.float32, tag="v0tt")
            nc.sync.dma_start(
                v0tt[:].rearrange("p (b j) -> p b j", b=BC),
                v0t[:].rearrange("b p j -> p b j"),
            )
            v03 = v0tt[:].rearrange("p (b j) -> p b j", b=BC)

            import contextlib
            _hints = (mybir.EngineType.DVE, mybir.EngineType.Activation,
                      mybir.EngineType.PE, mybir.EngineType.SP,
                      mybir.EngineType.Pool)
            loop_cm = tc.For_i(0, bench_iters, 1, hint_engines=_hints) \
                if bench_iters else contextlib.nullcontext()
            with loop_cm:
                for _ in range(CFG["unroll"] if bench_iters else 1):
                    body(nc, tc, x, y, v03, ut,
                         xin_p, rt_p, f_p, sp_p, cr_p, sm_p, pc_p,
                         sdt=sdt, skip=set(skip))
    nc.compile()
    return nc


def body(nc, tc, x, y, v03, ut,
         xin_p, rt_p, f_p, sp_p, cr_p, sm_p, pc_p, sdt, skip=()):
    AL = mybir.AluOpType
    AF = mybir.ActivationFunctionType
    LDN = CFG["ldn"]
    RLE = CFG["rle"]
    DFE = CFG["dfe"]
    STN = CFG["stn"]
    TMM = bool(CFG["tmm"])
    NL = BC * NDB
    state = {}
    bstate = {}

    def produce(g):
        t0 = g * G
        # joint tiles: [p, (b, c, d)]
        x4 = xin_p.tile([CH, BC * CPG * D], dt.float32, tag="xk")
        r4 = rt_p.tile([CH, BC * CPG * D], dt.float32, tag="rk")
        x5 = x4[:].rearrange("p (b c d) -> p b c d", b=BC, c=CPG)
        r5 = r4[:].rearrange("p (b c d) -> p b c d", b=BC, c=CPG)
        if "in" not in skip:
            xf = x4[:].rearrange("p (u d) -> p u d", d=D)
            for u0 in range(0, BC * CPG, LDN):
                _le = getattr(nc, ENGQ[CFG["ldq"][(g * ((BC * CPG) // LDN)
                                                   + u0 // LDN) % len(CFG["ldq"])]])
                _le.dma_start(
                    xf[:, u0:u0 + LDN, :].rearrange(
                        "p (b c) d -> p b c d", c=min(LDN, CPG)),
                    x[u0 // CPG:(u0 + LDN - 1) // CPG + 1,
                      t0 + (u0 % CPG) * CH:t0 + ((u0 + LDN - 1) % CPG + 1) * CH, :]
                    .rearrange("b (c p) d -> p b c d", p=CH))
        if "relu" not in skip:
            import contextlib
            rcm = tc.high_priority(offset=-CFG["rlp"]) if CFG["rlp"] \
                else contextlib.nullcontext()
            with rcm:
                for b in range(BC):
                    for c in range(CPG):
                        e = RLE[(g * BC * CPG + b * CPG + c) % len(RLE)]
                        if e == "a":
                            nc.scalar.activation(r5[:, b, c, :], x5[:, b, c, :],
                                                 AF.Relu, bias=0.0, scale=0.001)
                        else:
                            getattr(nc, ENGQ[e]).tensor_scalar(
                                r5[:, b, c, :], x5[:, b, c, :],
                                0.0, 0.001, op0=AL.max, op1=AL.mult)
        else:
            nc.vector.memset(r4[:], 0.0)
        state[g] = r5

    def consume(g):
        t0 = g * G
        r5 = state.pop(g)

        # joint carry tile: c3[:, c, :] = [b, j]-carry before chunk (g, c)
        carr = cr_p.tile([CH, (CPG + 1) * BC * NDB], dt.float32, tag="carr")
        c3 = carr[:].rearrange("p (c r) -> p c r", c=CPG + 1)
        with tc.high_priority():
            if g == 0:
                nc.vector.tensor_scalar(
                    c3[:, 0, :],
                    v03[:, :, :].rearrange("p b j -> p (b j)"),
                    -0.5, None, op0=AL.add)
                comp_old = sm_p.tile([CH, BC * NDB], dt.float32, tag="ckah")
                nc.vector.memset(comp_old[:], 0.0)
            elif "f" in skip and "kah" in skip:
                nc.vector.tensor_scalar(
                    c3[:, 0, :],
                    v03[:, :, :].rearrange("p b j -> p (b j)"),
                    -0.5, None, op0=AL.add)
                comp_old = bstate["comp"]
            else:
                nc.vector.tensor_copy(c3[:, 0, :], bstate["carr"][:, CPG, :])
                comp_old = bstate["comp"]

        # joint F tile: [p, (b, j, t)] with col 0 = prev chunk boundary
        # (hf8: F IS the output, int8, no boundary column)
        HF = CFG["odt"] == "hf8"
        fdt = {"f8": dt.bfloat16, "hf8": dt.int8}.get(CFG["odt"], dt.int16)
        fcols = G if HF else G + 1
        ft = f_p.tile([CH, BC * NDB * fcols], fdt, tag="fk")
        f3 = ft[:].rearrange("p (r t) -> p r t", t=fcols)
        f4 = ft[:].rearrange("p (b j t) -> p b j t", b=BC, j=NDB)
        fo = 0 if HF else 1
        if "diff" not in skip and not HF:
            with tc.high_priority():
                if g == 0:
                    nc.vector.memset(f3[:, :, 0], 0)
                else:
                    nc.vector.tensor_copy(f3[:, :, 0], bstate["f"][:, :, G])

        if CFG["odt"] == "hf8":
            s4 = s3 = spk = None
        elif CFG["odt"] == "p2":
            pk2 = sp_p.tile([CH, BC * NDB * (G // 4)], dt.int16, tag="pk2")
            pw4 = pk2[:].rearrange("p (b j t) -> p b j t", b=BC, j=NDB)
            pw3 = pk2[:].rearrange("p (r t) -> p r t", t=G // 4)
            s4 = s3 = spk = None
            if "diff" in skip and "out" not in skip:
                nc.vector.memset(pk2[:], 0)
        else:
            spk = sp_p.tile([CH, BC * NDB * G], sdt, tag="sk")
            s4 = spk[:].rearrange("p (b j t) -> p b j t", b=BC, j=NDB)
            s3 = spk[:].rearrange("p (r t) -> p r t", t=G)
            if "diff" in skip and "out" not in skip:
                nc.vector.memset(spk[:], 0)

        for c in range(CPG):
            # joint chunk tile: both batches side by side (4 PSUM banks)
            pck = pc_p.tile([CH, BC * D], dt.float32, tag="pck")
            if "mm" not in skip:
                for b in range(BC):
                    for j in range(NDB):
                        nc.tensor.matmul(
                            pck[:, b * D + j * CH:b * D + (j + 1) * CH],
                            r5[:, b, c, j * CH:(j + 1) * CH],
                            ut[:], start=True, stop=True,
                            is_transpose=TMM or None)
            elif g == 0 and c < CFG["pcb"]:
                nc.vector.memset(pck[:], 0.0)
            pc3 = pck[:].rearrange("p (r t) -> p r t", t=CH)

            with tc.high_priority():
                # F = rint(L + carr) = floor(c), int16, both batches
                if "f" not in skip:
                    cb = c3[:, c, :].unsqueeze(2).broadcast_to([CH, NL, CH])
                    nc.vector.scalar_tensor_tensor(
                        f3[:, :, fo + c * CH:fo + (c + 1) * CH],
                        pc3, 1.0, cb, op0=AL.mult, op1=AL.add)
                elif "diff" not in skip and c == 0:
                    nc.vector.memset(ft[:], 0)

                # joint Kahan carry update ([128, 16], one chain)
                if "kah" not in skip:
                    tot = pc3[:, :, CH - 1]
                    ykh = sm_p.tile([CH, BC * NDB], dt.float32, tag="ykah")
                    nc.vector.tensor_tensor(ykh[:], tot, comp_old[:],
                                            op=AL.subtract)
                    nc.vector.tensor_tensor(c3[:, c + 1, :], c3[:, c, :], ykh[:],
                                            op=AL.add)
                    dkh = sm_p.tile([CH, BC * NDB], dt.float32, tag="dkah")
                    nc.vector.tensor_tensor(dkh[:], c3[:, c + 1, :], c3[:, c, :],
                                            op=AL.subtract)
                    comp_new = sm_p.tile([CH, BC * NDB], dt.float32, tag="ckah")
                    nc.vector.tensor_tensor(comp_new[:], dkh[:], ykh[:],
                                            op=AL.subtract)
                    comp_old = comp_new
                elif "f" not in skip and c == 0:
                    for cc in range(CPG):
                        nc.vector.tensor_copy(c3[:, cc + 1, :], c3[:, cc, :])

            # spikes = F_t - F_{t-1} in {0,1} (joint, normal priority)
            if CFG["odt"] == "hf8":
                pass
            elif "diff" not in skip and CFG["odt"] == "p2":
                de = DFE[(g * CPG + c) % len(DFE)]
                spkc = sp_p.tile([CH, NL * CH], dt.int16, tag="skc")
                sc3 = spkc[:].rearrange("p (r t) -> p r t", t=CH)
                getattr(nc, ENGQ[de]).tensor_tensor(
                    sc3,
                    f3[:, :, 1 + c * CH:1 + (c + 1) * CH],
                    f3[:, :, c * CH:(c + 1) * CH], op=AL.subtract)
                e1 = CFG["pke"][(g * CPG + c) % len(CFG["pke"])]
                pk1 = sm_p.tile([CH, NL * (CH // 2)], dt.int16, tag="pk1")
                q3 = pk1[:].rearrange("p (r t) -> p r t", t=CH // 2)
                sce = sc3.rearrange("p r (t two) -> p r t two", two=2)
                getattr(nc, ENGQ[e1]).scalar_tensor_tensor(
                    q3, sce[:, :, :, 1], 2.0, sce[:, :, :, 0],
                    op0=AL.mult, op1=AL.add)
                e2 = CFG["pke"][(g * CPG + c + 1) % len(CFG["pke"])]
                q3e = q3.rearrange("p r (t two) -> p r t two", two=2)
                getattr(nc, ENGQ[e2]).scalar_tensor_tensor(
                    pw3[:, :, c * (CH // 4):(c + 1) * (CH // 4)],
                    q3e[:, :, :, 1], 4.0, q3e[:, :, :, 0],
                    op0=AL.mult, op1=AL.add)
            elif "diff" not in skip:
                de = DFE[(g * CPG + c) % len(DFE)]
                getattr(nc, ENGQ[de]).tensor_tensor(
                    s3[:, :, c * CH:(c + 1) * CH],
                    f3[:, :, 1 + c * CH:1 + (c + 1) * CH],
                    f3[:, :, c * CH:(c + 1) * CH], op=AL.subtract)

        bstate["carr"] = c3
        bstate["comp"] = comp_old
        bstate["f"] = f3

        if CFG["odt"] == "p2":
            if "out" not in skip:
                for u0 in range(BC):
                    nc.gpsimd.dma_start(
                        y[u0, :, t0 // 4:(t0 + G) // 4]
                        .rearrange("(j p) t -> p j t", p=CH),
                        pw4[:, u0, :, :])
            return

        if "out" in skip:
            return
        cast = CFG["odt"] == "i16c8"
        src4 = f4 if CFG["odt"] == "hf8" else s4
        for u0 in range(0, BC * CPG, STN):
            qc = CFG["stq"][(g * ((BC * CPG) // STN) + u0 // STN) % len(CFG["stq"])]
            _se = nc.gpsimd if cast else getattr(nc, ENGQ[qc])
            b0, b1 = u0 // CPG, (u0 + STN - 1) // CPG + 1
            cl, ch_ = u0 % CPG, (u0 + STN - 1) % CPG + 1
            _se.dma_start(
                y[b0:b1, :, t0 + cl * CH:t0 + ch_ * CH]
                .rearrange("b (j p) t -> p b j t", p=CH),
                src4[:, b0:b1, :, cl * CH:ch_ * CH])

    for s in range(NG + 1):
        if s < NG:
            produce(s)
        if s >= 1:
            consume(s - 1)


def _get_nc():
    if "nc" not in _cache:
        _cache["nc"] = build_nc()
    return _cache["nc"]


def _make_in_maps(x, v0):
    uv = np.triu(np.ones((CH, CH), dtype=np.float32))
    in_maps = []
    for c in range(N_CORES):
        xb = np.ascontiguousarray(x[BC * c:BC * (c + 1)])
        v0b = v0[BC * c:BC * (c + 1)]
        v0tb = np.ascontiguousarray(
            v0b.reshape(BC, NDB, CH).transpose(0, 2, 1).astype(np.float32))
        in_maps.append({"x": xb, "v0t": v0tb, "u": uv})
    return in_maps


def _get_runner():
    """Build (once) a cached jitted SPMD executable over the 8 cores."""
    if "runner" in _cache:
        return _cache["runner"]
    import jax
    from jax.sharding import Mesh, PartitionSpec
    from jax.experimental.shard_map import shard_map
    from concourse import bass2jax

    nc = _get_nc()
    bass2jax.install_neuronx_cc_hook()
    partition_name = nc.partition_id_tensor.name if nc.partition_id_tensor else None
    in_names, out_names, out_avals = [], [], []
    for alloc in nc.m.functions[0].allocations:
        if not isinstance(alloc, mybir.MemoryLocationSet):
            continue
        name = alloc.memorylocations[0].name
        if alloc.kind == "ExternalInput":
            if name != partition_name:
                in_names.append(name)
        elif alloc.kind == "ExternalOutput":
            out_names.append(name)
            out_avals.append(jax.core.ShapedArray(
                tuple(alloc.tensor_shape), dt.np(alloc.dtype)))
    n_params = len(in_names)
    zero_outs = [np.zeros(a.shape, a.dtype) for a in out_avals]
    all_names = in_names + out_names + ([partition_name] if partition_name else [])

    def _body(*args):
        operands = list(args)
        if partition_name is not None:
            operands.append(bass2jax.partition_id_tensor())
        return tuple(bass2jax._bass_exec_p.bind(
            *operands, out_avals=tuple(out_avals), in_names=tuple(all_names),
            out_names=tuple(out_names), lowering_input_output_aliases=(),
            sim_require_finite=True, sim_require_nnan=True, nc=nc))

    devices = jax.devices()[:N_CORES]
    mesh = Mesh(np.asarray(devices), ("core",))
    nin = n_params + len(out_names)
    fn = jax.jit(shard_map(_body, mesh=mesh,
                           in_specs=(PartitionSpec("core"),) * nin,
                           out_specs=(PartitionSpec("core"),) * len(out_names),
                           check_rep=False))
    _cache["runner"] = (fn, in_names, out_names, zero_outs)
    return _cache["runner"]


def kernel(inputs, initial_state):
    import jax
    x = np.ascontiguousarray(np.asarray(inputs, dtype=np.float32))
    v0 = np.ascontiguousarray(np.asarray(initial_state, dtype=np.float32))
    assert x.shape == (B, T, D) and v0.shape == (B, D)
    fn, in_names, out_names, zero_outs = _get_runner()
    in_maps = _make_in_maps(x, v0)
    concat_in = [np.concatenate([np.asarray(in_maps[c][nm])
                                 for c in range(N_CORES)], axis=0)
                 for nm in in_names]
    concat_zero = [np.concatenate([z] * N_CORES, axis=0) for z in zero_outs]
    outs = jax.block_until_ready(fn(*concat_in, *concat_zero))
    ydt = np.asarray(outs[out_names.index("y")])        # [B, D, T(/4)]
    scale = np.float32(1.0) / np.float32(0.001)         # matches reference /DT
    if CFG["odt"] == "f8":
        ydt = (ydt.view(np.uint8) != 0).astype(np.int8)
    if CFG["odt"] == "hf8":
        # ydt holds F = floor(c) per [B, D, T]; spikes = adjacent diff
        # (F_{-1} = floor(v0) = 0 since v0 in [0,1))
        ydt = np.diff(ydt, axis=2, prepend=np.int8(0))
    if CFG["odt"] == "p2":
        yb = ydt.view(np.uint8)
        bits = np.stack([(yb >> k) & 1 for k in range(4)], axis=-1)
        ydt = bits.reshape(B, D, T)
    out = ydt.astype(np.float32).transpose(0, 2, 1) * scale
    return np.ascontiguousarray(out)
